# revision 7
# baseline (speedup 1.0000x reference)
"""Batched int8 GEMM C = A @ B^T with fused three-scale dequant/requant.

H=16 heads sharded 2-per-core across 8 NeuronCores. Per head:
  acc[m,n] = sum_k A[m,k] * B[n,k]          (ints 0..126, exact)
  C = clip(round(acc * sA[m] * sB[n] / sO[m]), -127, 127)

Device-side math:
  - scale_B is folded into B on the host: b' = fp16(B * sB[n]) (the PE
    upcasts fp16 to e10m11 so products are exact; only the fold rounds,
    rel err <= 2^-12).
  - matmul a'[k,m].T @ b'[k,n] accumulates acc*sB in fp32 PSUM.
  - one fused drain per tile: int8_out = convert(psum * u[m]) with
    u = sA/sO as a per-partition scalar. The f32->int8 convert is
    round-to-nearest-even + saturating, matching round+clip exactly.
    The drain is split column-wise between ACT and DVE so both engines
    run concurrently and finish together (~1.08us each per tile).
"""

import numpy as np

H, M, N, K = 16, 2048, 2048, 128
NCORES = 8
HPC = H // NCORES  # heads per core
MT = M // 128  # m-tiles per head
NQ = N // 512  # n-chunks per m-tile
XSPLIT = 1072  # ACT drains [0:XSPLIT), DVE [XSPLIT:N) -- equal ns per engine

_compiled = None


def _build():
    import concourse.bacc as bacc
    import concourse.mybir as mybir
    import concourse.tile as tile

    nc = bacc.Bacc("TRN2", target_bir_lowering=False, debug=False)

    a_d = nc.dram_tensor("a", (HPC, K, M), mybir.dt.float16, kind="ExternalInput")
    b_d = nc.dram_tensor("b", (HPC, K, N), mybir.dt.float16, kind="ExternalInput")
    u_d = nc.dram_tensor("u", (HPC, 128, MT), mybir.dt.float32, kind="ExternalInput")
    c_d = nc.dram_tensor("c", (HPC, M, N), mybir.dt.int8, kind="ExternalOutput")

    with tile.TileContext(nc) as tc:
        with (
            tc.tile_pool(name="am", bufs=2) as a_pool,
            tc.tile_pool(name="bm", bufs=2) as b_pool,
            tc.tile_pool(name="us", bufs=2) as u_pool,
            tc.tile_pool(name="out", bufs=4) as out_pool,
            tc.tile_pool(name="ps", bufs=2, space="PSUM") as ps_pool,
        ):
            for h in range(HPC):
                # chunked loads, interleaved so the first matmul starts early
                u_sb = u_pool.tile([128, MT], mybir.dt.float32, tag="u")
                nc.sync.dma_start(u_sb[:], u_d[h])
                a_sb = [
                    a_pool.tile([128, 512], mybir.dt.float16, tag=f"a{i}", name=f"a{h}_{i}")
                    for i in range(4)
                ]
                b_sb = [
                    b_pool.tile([128, 512], mybir.dt.float16, tag=f"b{q}", name=f"b{h}_{q}")
                    for q in range(NQ)
                ]
                order = [("b", 0), ("a", 0), ("b", 1), ("b", 2), ("b", 3),
                         ("a", 1), ("a", 2), ("a", 3)]
                for kind, i in order:
                    src = b_d if kind == "b" else a_d
                    dst = b_sb[i] if kind == "b" else a_sb[i]
                    nc.sync.dma_start(dst[:], src[h, :, i * 512 : (i + 1) * 512])
                for jj in range(MT // 2):
                    ot = out_pool.tile([128, 2 * N], mybir.dt.int8, tag="ot")
                    for sub in range(2):
                        j = 2 * jj + sub
                        ps = ps_pool.tile([128, N], mybir.dt.float32, tag="ps")
                        aj = a_sb[j // 4][:, (j % 4) * 128 : (j % 4 + 1) * 128]
                        for q in range(NQ):
                            nc.tensor.matmul(
                                ps[:, q * 512 : (q + 1) * 512],
                                aj,
                                b_sb[q][:],
                                start=True,
                                stop=True,
                            )
                        off = sub * N
                        usc = u_sb[:, j : j + 1]
                        nc.scalar.activation(
                            ot[:, off : off + XSPLIT],
                            ps[:, 0:XSPLIT],
                            mybir.ActivationFunctionType.Copy,
                            scale=usc,
                        )
                        nc.vector.tensor_scalar_mul(
                            ot[:, off + XSPLIT : off + N], ps[:, XSPLIT:N], usc
                        )
                    # one DMA moves both m-tiles' int8 rows out
                    dst = c_d[h, 2 * jj * 128 : (2 * jj + 2) * 128, :].rearrange(
                        "(s p) n -> p s n", s=2
                    )
                    nc.sync.dma_start(dst, ot[:].rearrange("p (s n) -> p s n", s=2))

    nc.compile()
    return nc


def _get_compiled():
    global _compiled
    if _compiled is None:
        _compiled = _build()
    return _compiled


def _prep_inputs(A, B, scale_A, scale_B, scale_out):
    # u = sA/sO in fp32 (same divide the reference does), laid out so
    # column j holds the per-partition scalars of m-tile j.
    u = (scale_A / scale_out).astype(np.float32)  # [H, M]
    u_t = np.ascontiguousarray(
        u.reshape(H, MT, 128).transpose(0, 2, 1)
    )  # [H, 128, MT]

    a_t = np.ascontiguousarray(A.transpose(0, 2, 1)).astype(np.float16)  # [H,K,M]
    bv = B.astype(np.float64) * scale_B.astype(np.float64)[:, :, None]
    b_t = np.ascontiguousarray(bv.astype(np.float16).transpose(0, 2, 1))  # [H,K,N]
    return a_t, b_t, u_t


def _in_maps(a_t, b_t, u_t):
    return [
        {
            "a": np.ascontiguousarray(a_t[i * HPC : (i + 1) * HPC]),
            "b": np.ascontiguousarray(b_t[i * HPC : (i + 1) * HPC]),
            "u": np.ascontiguousarray(u_t[i * HPC : (i + 1) * HPC]),
        }
        for i in range(NCORES)
    ]


def kernel(A, B, scale_A, scale_B, scale_out):
    from concourse.bass_utils import run_bass_kernel_spmd

    nc = _get_compiled()
    a_t, b_t, u_t = _prep_inputs(A, B, scale_A, scale_B, scale_out)
    res = run_bass_kernel_spmd(nc, _in_maps(a_t, b_t, u_t), core_ids=list(range(NCORES)))
    C = np.empty((H, M, N), dtype=np.int32)
    for i in range(NCORES):
        C[i * HPC : (i + 1) * HPC] = res.results[i]["c"].astype(np.int32)
    return (C, np.asarray(scale_out))


# revision 8
# speedup vs baseline: 1.0095x; 1.0095x over previous
"""Batched int8 GEMM C = A @ B^T with fused three-scale dequant/requant.

H=16 heads sharded 2-per-core across 8 NeuronCores. Per head:
  acc[m,n] = sum_k A[m,k] * B[n,k]          (ints 0..126, exact)
  C = clip(round(acc * sA[m] * sB[n] / sO[m]), -127, 127)

Device-side math:
  - scale_B is folded into B on the host: b' = fp16(B * sB[n]) (the PE
    upcasts fp16 to e10m11 so products are exact; only the fold rounds,
    rel err <= 2^-12).
  - matmul a'[k,m].T @ b'[k,n] accumulates acc*sB in fp32 PSUM.
  - one fused drain per tile: int8_out = convert(psum * u[m]) with
    u = sA/sO as a per-partition scalar. The f32->int8 convert is
    round-to-nearest-even + saturating, matching round+clip exactly.
    The drain is split column-wise between ACT and DVE so both engines
    run concurrently and finish together (~1.08us each per tile).
"""

import numpy as np

H, M, N, K = 16, 2048, 2048, 128
NCORES = 8
HPC = H // NCORES  # heads per core
MT = M // 128  # m-tiles per head
NQ = N // 512  # n-chunks per m-tile
XSPLIT = 1024  # ACT drains [0:XSPLIT), DVE [XSPLIT:N) -- equal ns per engine

_compiled = None


def _build():
    import concourse.bacc as bacc
    import concourse.mybir as mybir
    import concourse.tile as tile

    nc = bacc.Bacc("TRN2", target_bir_lowering=False, debug=False)

    a_d = nc.dram_tensor("a", (HPC, K, M), mybir.dt.float16, kind="ExternalInput")
    b_d = nc.dram_tensor("b", (HPC, K, N), mybir.dt.float16, kind="ExternalInput")
    u_d = nc.dram_tensor("u", (HPC, 128, MT), mybir.dt.float32, kind="ExternalInput")
    c_d = nc.dram_tensor("c", (HPC, M, N), mybir.dt.int8, kind="ExternalOutput")

    with tile.TileContext(nc) as tc:
        with (
            tc.tile_pool(name="am", bufs=2) as a_pool,
            tc.tile_pool(name="bm", bufs=2) as b_pool,
            tc.tile_pool(name="us", bufs=2) as u_pool,
            tc.tile_pool(name="out", bufs=4) as out_pool,
            tc.tile_pool(name="ps", bufs=2, space="PSUM") as ps_pool,
        ):
            for h in range(HPC):
                # chunked loads, interleaved so the first matmul starts early
                u_sb = u_pool.tile([128, MT], mybir.dt.float32, tag="u")
                nc.sync.dma_start(u_sb[:], u_d[h])
                a_sb = [
                    a_pool.tile([128, 512], mybir.dt.float16, tag=f"a{i}", name=f"a{h}_{i}")
                    for i in range(4)
                ]
                b_sb = [
                    b_pool.tile([128, 512], mybir.dt.float16, tag=f"b{q}", name=f"b{h}_{q}")
                    for q in range(NQ)
                ]
                order = [("b", 0), ("a", 0), ("b", 1), ("b", 2), ("b", 3),
                         ("a", 1), ("a", 2), ("a", 3)]
                for kind, i in order:
                    src = b_d if kind == "b" else a_d
                    dst = b_sb[i] if kind == "b" else a_sb[i]
                    nc.sync.dma_start(dst[:], src[h, :, i * 512 : (i + 1) * 512])
                for jj in range(MT // 2):
                    ot = out_pool.tile([128, 2 * N], mybir.dt.int8, tag="ot")
                    for sub in range(2):
                        j = 2 * jj + sub
                        ps = ps_pool.tile([128, N], mybir.dt.float32, tag="ps")
                        aj = a_sb[j // 4][:, (j % 4) * 128 : (j % 4 + 1) * 128]
                        for q in range(NQ):
                            nc.tensor.matmul(
                                ps[:, q * 512 : (q + 1) * 512],
                                aj,
                                b_sb[q][:],
                                start=True,
                                stop=True,
                            )
                        off = sub * N
                        usc = u_sb[:, j : j + 1]
                        nc.scalar.activation(
                            ot[:, off : off + XSPLIT],
                            ps[:, 0:XSPLIT],
                            mybir.ActivationFunctionType.Copy,
                            scale=usc,
                        )
                        nc.vector.tensor_scalar_mul(
                            ot[:, off + XSPLIT : off + N], ps[:, XSPLIT:N], usc
                        )
                    # one DMA moves both m-tiles' int8 rows out
                    dst = c_d[h, 2 * jj * 128 : (2 * jj + 2) * 128, :].rearrange(
                        "(s p) n -> p s n", s=2
                    )
                    nc.sync.dma_start(dst, ot[:].rearrange("p (s n) -> p s n", s=2))

    nc.compile()
    return nc


def _get_compiled():
    global _compiled
    if _compiled is None:
        _compiled = _build()
    return _compiled


def _prep_inputs(A, B, scale_A, scale_B, scale_out):
    # u = sA/sO in fp32 (same divide the reference does), laid out so
    # column j holds the per-partition scalars of m-tile j.
    u = (scale_A / scale_out).astype(np.float32)  # [H, M]
    u_t = np.ascontiguousarray(
        u.reshape(H, MT, 128).transpose(0, 2, 1)
    )  # [H, 128, MT]

    a_t = np.ascontiguousarray(A.transpose(0, 2, 1)).astype(np.float16)  # [H,K,M]
    bv = B.astype(np.float64) * scale_B.astype(np.float64)[:, :, None]
    b_t = np.ascontiguousarray(bv.astype(np.float16).transpose(0, 2, 1))  # [H,K,N]
    return a_t, b_t, u_t


def _in_maps(a_t, b_t, u_t):
    return [
        {
            "a": np.ascontiguousarray(a_t[i * HPC : (i + 1) * HPC]),
            "b": np.ascontiguousarray(b_t[i * HPC : (i + 1) * HPC]),
            "u": np.ascontiguousarray(u_t[i * HPC : (i + 1) * HPC]),
        }
        for i in range(NCORES)
    ]


def kernel(A, B, scale_A, scale_B, scale_out):
    from concourse.bass_utils import run_bass_kernel_spmd

    nc = _get_compiled()
    a_t, b_t, u_t = _prep_inputs(A, B, scale_A, scale_B, scale_out)
    res = run_bass_kernel_spmd(nc, _in_maps(a_t, b_t, u_t), core_ids=list(range(NCORES)))
    C = np.empty((H, M, N), dtype=np.int32)
    for i in range(NCORES):
        C[i * HPC : (i + 1) * HPC] = res.results[i]["c"].astype(np.int32)
    return (C, np.asarray(scale_out))


# revision 9
# speedup vs baseline: 1.0359x; 1.0261x over previous
"""Batched int8 GEMM C = A @ B^T with fused three-scale dequant/requant.

H=16 heads sharded 2-per-core across 8 NeuronCores. Per head:
  acc[m,n] = sum_k A[m,k] * B[n,k]          (ints 0..126, exact)
  C = clip(round(acc * sA[m] * sB[n] / sO[m]), -127, 127)

Device-side math:
  - scale_B is folded into B on the host: b' = fp16(B * sB[n]) (the PE
    upcasts fp16 to e10m11 so products are exact; only the fold rounds,
    rel err <= 2^-12).
  - matmul a'[k,m].T @ b'[k,n] accumulates acc*sB in fp32 PSUM.
  - one fused drain per tile: int8_out = convert(psum * u[m]) with
    u = sA/sO as a per-partition scalar. The f32->int8 convert is
    round-to-nearest-even + saturating, matching round+clip exactly.
    The drain is split column-wise between ACT and DVE so both engines
    run concurrently and finish together (~1.08us each per tile).
"""

import numpy as np

H, M, N, K = 16, 2048, 2048, 128
NCORES = 8
HPC = H // NCORES  # heads per core
MT = M // 128  # m-tiles per head
NQ = N // 512  # n-chunks per m-tile
XSPLIT = 1024  # ACT drains [0:XSPLIT), DVE [XSPLIT:N) -- equal ns per engine

_compiled = None


def _build():
    import concourse.bacc as bacc
    import concourse.mybir as mybir
    import concourse.tile as tile

    nc = bacc.Bacc("TRN2", target_bir_lowering=False, debug=False)

    a_d = nc.dram_tensor("a", (HPC, K, M), mybir.dt.float16, kind="ExternalInput")
    b_d = nc.dram_tensor("b", (HPC, K, N), mybir.dt.float16, kind="ExternalInput")
    u_d = nc.dram_tensor("u", (HPC, 128, MT), mybir.dt.float32, kind="ExternalInput")
    c_d = nc.dram_tensor("c", (HPC, M, N), mybir.dt.int8, kind="ExternalOutput")

    with tile.TileContext(nc) as tc:
        with (
            tc.tile_pool(name="am", bufs=2) as a_pool,
            tc.tile_pool(name="bm", bufs=2) as b_pool,
            tc.tile_pool(name="us", bufs=2) as u_pool,
            tc.tile_pool(name="out", bufs=4) as out_pool,
            tc.tile_pool(name="ps", bufs=2, space="PSUM") as ps_pool,
        ):
            for h in range(HPC):
                # chunked loads, interleaved so the first matmul starts early
                u_sb = u_pool.tile([128, MT], mybir.dt.float32, tag="u")
                nc.sync.dma_start(u_sb[:], u_d[h])
                a_sb = [
                    a_pool.tile([128, 512], mybir.dt.float16, tag=f"a{i}", name=f"a{h}_{i}")
                    for i in range(4)
                ]
                b_sb = [
                    b_pool.tile([128, 512], mybir.dt.float16, tag=f"b{q}", name=f"b{h}_{q}")
                    for q in range(NQ)
                ]
                order = [("b", 0), ("a", 0), ("b", 1), ("b", 2), ("b", 3),
                         ("a", 1), ("a", 2), ("a", 3)]
                for kind, i in order:
                    src = b_d if kind == "b" else a_d
                    dst = b_sb[i] if kind == "b" else a_sb[i]
                    nc.sync.dma_start(dst[:], src[h, :, i * 512 : (i + 1) * 512])
                YS = N - XSPLIT
                for jj in range(MT // 2):
                    # separate out tiles per engine -- a shared tile would
                    # serialize ACT and DVE on a false write dependency
                    ot_a = out_pool.tile(
                        [128, 2 * XSPLIT], mybir.dt.int8, tag="ota", name=f"ota{h}_{jj}"
                    )
                    ot_v = out_pool.tile(
                        [128, 2 * YS], mybir.dt.int8, tag="otv", name=f"otv{h}_{jj}"
                    )
                    for sub in range(2):
                        j = 2 * jj + sub
                        ps = ps_pool.tile([128, N], mybir.dt.float32, tag="ps")
                        aj = a_sb[j // 4][:, (j % 4) * 128 : (j % 4 + 1) * 128]
                        for q in range(NQ):
                            nc.tensor.matmul(
                                ps[:, q * 512 : (q + 1) * 512],
                                aj,
                                b_sb[q][:],
                                start=True,
                                stop=True,
                            )
                        usc = u_sb[:, j : j + 1]
                        nc.scalar.activation(
                            ot_a[:, sub * XSPLIT : (sub + 1) * XSPLIT],
                            ps[:, 0:XSPLIT],
                            mybir.ActivationFunctionType.Copy,
                            scale=usc,
                        )
                        nc.vector.tensor_scalar_mul(
                            ot_v[:, sub * YS : (sub + 1) * YS], ps[:, XSPLIT:N], usc
                        )
                    rows = c_d[h, 2 * jj * 128 : (2 * jj + 2) * 128, :]
                    dst_a = rows[:, 0:XSPLIT].rearrange("(s p) n -> p s n", s=2)
                    dst_v = rows[:, XSPLIT:N].rearrange("(s p) n -> p s n", s=2)
                    nc.sync.dma_start(
                        dst_a, ot_a[:].rearrange("p (s n) -> p s n", s=2)
                    )
                    nc.sync.dma_start(
                        dst_v, ot_v[:].rearrange("p (s n) -> p s n", s=2)
                    )

    nc.compile()
    return nc


def _get_compiled():
    global _compiled
    if _compiled is None:
        _compiled = _build()
    return _compiled


def _prep_inputs(A, B, scale_A, scale_B, scale_out):
    # u = sA/sO in fp32 (same divide the reference does), laid out so
    # column j holds the per-partition scalars of m-tile j.
    u = (scale_A / scale_out).astype(np.float32)  # [H, M]
    u_t = np.ascontiguousarray(
        u.reshape(H, MT, 128).transpose(0, 2, 1)
    )  # [H, 128, MT]

    a_t = np.ascontiguousarray(A.transpose(0, 2, 1)).astype(np.float16)  # [H,K,M]
    bv = B.astype(np.float64) * scale_B.astype(np.float64)[:, :, None]
    b_t = np.ascontiguousarray(bv.astype(np.float16).transpose(0, 2, 1))  # [H,K,N]
    return a_t, b_t, u_t


def _in_maps(a_t, b_t, u_t):
    return [
        {
            "a": np.ascontiguousarray(a_t[i * HPC : (i + 1) * HPC]),
            "b": np.ascontiguousarray(b_t[i * HPC : (i + 1) * HPC]),
            "u": np.ascontiguousarray(u_t[i * HPC : (i + 1) * HPC]),
        }
        for i in range(NCORES)
    ]


def kernel(A, B, scale_A, scale_B, scale_out):
    from concourse.bass_utils import run_bass_kernel_spmd

    nc = _get_compiled()
    a_t, b_t, u_t = _prep_inputs(A, B, scale_A, scale_B, scale_out)
    res = run_bass_kernel_spmd(nc, _in_maps(a_t, b_t, u_t), core_ids=list(range(NCORES)))
    C = np.empty((H, M, N), dtype=np.int32)
    for i in range(NCORES):
        C[i * HPC : (i + 1) * HPC] = res.results[i]["c"].astype(np.int32)
    return (C, np.asarray(scale_out))
